# revision 6
# baseline (speedup 1.0000x reference)
"""Trainium2 Bass kernel for causal self-attention (B=2, S=2048, D=1024, H=16).

Sharding: 8 cores = 2 batches x 4 head-groups. Core c handles batch c//4 and
heads 4*(c%4) .. 4*(c%4)+4. Each core receives its batch's x [2048, 1024] and
its [1024, 768] slice of w_qkv (q/k/v columns for its 4 heads), and produces
the [2048, 256] output slice. No cross-core communication is needed; the host
gathers the slices. w_o is unused by the reference (no output projection).

Per-core kernel (Tile framework):
  1. x -> SBUF, PE-transpose into xT [d, s] chunks (float32r).
  2. Projection with w as the stationary operand produces qT/kT/vT [cols, s]
     directly (transposed layouts are what attention needs).
  3. vT is re-transposed to natural v [s, hd] and augmented with a ones
     column (vaug) so the AV matmul also produces softmax denominators.
  4. Attention per head pair, per 512-wide query chunk: scores are computed
     transposed ST[j, i] = k_j . q_i with two heads packed into the 128-row
     PE array (K=64 each); exp on ACT over [128, 2048] PSUM groups; causal
     mask = affine_select on the diagonal 128x128 block + restricted AV
     column ranges; AV accumulates outT [65, i] with v||ones stationary; a
     final PE transpose + reciprocal*mul yields the natural-layout output.

Matmuls run in float32r (full PE speed at N>=256, ~TF32 precision); psum and
the softmax arithmetic are fp32.
"""

import sys

sys.path.insert(0, "/opt/trn_rl_repo")

from contextlib import ExitStack

import numpy as np

import concourse.bass as bass
import concourse.tile as tile
from concourse import bacc, masks, mybir
from concourse.bass_utils import run_bass_kernel_spmd

B, S, D, H = 2, 2048, 1024, 16
HD = 64          # head dim
HPC = 4          # heads per core
NCORES = 8
P = 128
NS = S // P      # 16 s-blocks
KC = D // P      # 8 d-chunks
CH = 512         # query-chunk width
NT = S // CH     # 4 query chunks
COLS = 3 * HPC * HD   # 768 projection columns per core
NM = COLS // P        # 6 projection m-tiles (q01,q23,k01,k23,v01,v23)
F32 = mybir.dt.float32
F32R = mybir.dt.float32r
SCALE = 1.0 / np.sqrt(HD)

PSUM = bass.MemorySpace.PSUM


def _build_body(ctx: ExitStack, tc: "tile.TileContext", x_d, w_d, o_d):
    nc = tc.nc

    persist = ctx.enter_context(tc.tile_pool(name="persist", bufs=1))
    ident = persist.tile([P, P], F32)
    masks.make_identity(nc, ident[:])

    # v in natural layout + ones column, per (j-block, head): [128, 65] slices
    vaug = persist.tile([P, NS * HPC * 65], F32R)
    ones_col = persist.tile([P, 1], F32)
    nc.vector.memset(ones_col[:], 1.0)
    # memset can't write f32r; a DVE copy can (and rounds).
    nc.vector.tensor_copy(
        vaug[:].rearrange("p (n c) -> p n c", c=65)[:, :, 64:65],
        ones_col[:].broadcast_to([P, NS * HPC, 1]),
    )
    # final output staging [128, 16 i-blocks * 4 heads * 64]
    out_sb = persist.tile([P, NS * HPC * HD], F32)

    with (
        tc.tile_pool(name="w", bufs=1) as wp,
        tc.tile_pool(name="xT", bufs=1) as xtp,
    ):
        w_all = wp.tile([P, KC * COLS], F32R)     # [128, 8*768]
        nc.sync.dma_start(
            w_all[:].rearrange("p (k c) -> p k c", k=KC),
            w_d.rearrange("(k p) c -> p k c", p=P),
        )
        xT = xtp.tile([P, KC * S], F32R)          # [128, 8*2048]

        # ---- Phase A1: load x and transpose to xT --------------------------
        with (
            tc.tile_pool(name="xnat", bufs=8) as xnp,
            tc.tile_pool(name="ps_tp", bufs=2, space=PSUM) as ps_tp,
        ):
            for ig in range(NS // 4):  # groups of 4 s-blocks
                xts = []
                for b in range(4):
                    i = ig * 4 + b
                    t = xnp.tile([P, D], F32, tag="xn")
                    nc.sync.dma_start(t[:], x_d[i * P:(i + 1) * P, :])
                    xts.append(t)
                for k in range(KC):
                    ps = ps_tp.tile([P, 512], F32, tag="tp")
                    for b in range(4):
                        nc.tensor.transpose(
                            ps[:, b * P:(b + 1) * P],
                            xts[b][:, k * P:(k + 1) * P],
                            ident[:],
                        )
                    # DVE is idle in phase A; writes round fp32 -> f32r
                    nc.vector.tensor_copy(
                        xT[:, k * S + ig * 512: k * S + (ig + 1) * 512], ps[:]
                    )

        # ---- Phases A2+B per head pair ------------------------------------
        with (
            tc.tile_pool(name="qk", bufs=1) as qkp,
            tc.tile_pool(name="vT", bufs=1) as vtp_pool,
            tc.tile_pool(name="ps_small", bufs=2, space=PSUM) as ps_small,
            tc.tile_pool(name="ps_st", bufs=1, space=PSUM) as ps_st,
            tc.tile_pool(name="ps_o", bufs=2, space=PSUM) as ps_o,
            tc.tile_pool(name="p", bufs=2) as pp,
            tc.tile_pool(name="osb", bufs=2) as osbp,
            tc.tile_pool(name="rcol", bufs=4) as rcp,
        ):
            qkT = qkp.tile([P, 4 * S], F32R)      # m0,m1 = q(h01,h23); m2,m3 = k
            vT = vtp_pool.tile([P, 2 * S], F32)   # v pairs (h01, h23)

            def proj(m, dest, dcol, copy_eng):
                """dest[:, dcol*S + s] = (w col-block m)^T @ x^T."""
                for sblk in range(NT):
                    pp_ps = ps_small.tile([P, 512], F32, tag="small")
                    for k in range(KC):
                        nc.tensor.matmul(
                            pp_ps[:],
                            w_all[:, k * COLS + m * P: k * COLS + (m + 1) * P],
                            xT[:, k * S + sblk * 512: k * S + (sblk + 1) * 512],
                            start=(k == 0),
                            stop=(k == KC - 1),
                        )
                    dst = dest[:, dcol * S + sblk * 512: dcol * S + (sblk + 1) * 512]
                    if copy_eng == "act":
                        nc.scalar.copy(dst, pp_ps[:])
                    else:
                        nc.vector.tensor_copy(dst, pp_ps[:])

            def vtranspose(pair):
                """vT pair -> natural v in vaug slices (+ ones col preset)."""
                h0, h1 = 2 * pair, 2 * pair + 1
                for i in range(NS):
                    tp_ps = ps_small.tile([P, 512], F32, tag="small")
                    nc.tensor.transpose(
                        tp_ps[:, 0:P],
                        vT[:, pair * S + i * P: pair * S + (i + 1) * P],
                        ident[:],
                    )
                    nc.vector.tensor_copy(
                        vaug[:, (i * HPC + h0) * 65: (i * HPC + h0) * 65 + 64],
                        tp_ps[:, 0:64],
                    )
                    nc.vector.tensor_copy(
                        vaug[:, (i * HPC + h1) * 65: (i * HPC + h1) * 65 + 64],
                        tp_ps[:, 64:P],
                    )

            out_view = out_sb[:].rearrange("p (i g d) -> p i g d", g=HPC, d=HD)

            def attn(pair, t):
                """Heads 2*pair, 2*pair+1; query chunk t (i in [512t, 512t+512))."""
                hA, hB = 2 * pair, 2 * pair + 1
                qm, km = pair, 2 + pair
                po_a = ps_o.tile([65, 512], F32, tag="o")
                po_b = ps_o.tile([65, 512], F32, tag="o")
                po = {hA: po_a, hB: po_b}
                for jg in range(2 * (t + 1)):  # groups of 2 j-blocks
                    st = ps_st.tile([P, 2048], F32, tag="st")
                    # ST[j, i] = k_j . q_i ; two heads packed in rows 0-63/64-127
                    for b in range(2):
                        jb = 2 * jg + b
                        for hi, h in enumerate((hA, hB)):
                            hb = (h % 2) * 64
                            nc.tensor.matmul(
                                st[:, (hi * 2 + b) * 512: (hi * 2 + b + 1) * 512],
                                qkT[hb:hb + 64, km * S + jb * P: km * S + (jb + 1) * P],
                                qkT[hb:hb + 64, qm * S + t * 512: qm * S + (t + 1) * 512],
                                start=True,
                                stop=True,
                            )
                    p_t = pp.tile([P, 2048], F32R, tag="p")
                    nc.scalar.activation(
                        p_t[:], st[:], mybir.ActivationFunctionType.Exp,
                        scale=float(SCALE),
                    )
                    # triangular mask on the diagonal 128x128 sub-block only;
                    # the fully-masked zone is skipped by AV column ranges.
                    for b in range(2):
                        doff = 2 * jg + b - 4 * t
                        if doff >= 0:
                            for hi in range(2):
                                c0 = (hi * 2 + b) * 512 + 128 * doff
                                sl = p_t[:, c0:c0 + 128]
                                nc.gpsimd.affine_select(
                                    out=sl,
                                    in_=sl,
                                    compare_op=mybir.AluOpType.is_ge,
                                    fill=0.0,
                                    base=0,
                                    channel_multiplier=-1,
                                    pattern=[[1, 128]],
                                )
                    # AV accumulate: outT[d|1, i] += vaug^T @ p  (causal col range)
                    for b in range(2):
                        jb = 2 * jg + b
                        off = max(0, 128 * (jb - 4 * t))
                        for hi, h in enumerate((hA, hB)):
                            nc.tensor.matmul(
                                po[h][:, off:512],
                                vaug[:, (jb * HPC + h) * 65: (jb * HPC + h + 1) * 65],
                                p_t[:, (hi * 2 + b) * 512 + off: (hi * 2 + b + 1) * 512],
                                start=(jg == 0 and b == 0),
                                stop=(jg == 2 * t + 1 and b == 1),
                            )
                # finalize: transpose outT to natural, divide by denominator
                for h in (hA, hB):
                    osb_t = osbp.tile([65, 512], F32, tag="osb")
                    nc.vector.tensor_copy(osb_t[:], po[h][:])
                    fin = ps_small.tile([P, 512], F32, tag="small")
                    for b in range(4):
                        nc.tensor.transpose(
                            fin[:, b * 65:(b + 1) * 65],
                            osb_t[:, b * P:(b + 1) * P],
                            ident[0:65, 0:65],
                        )
                    fin_view = fin[:, 0:260].rearrange("p (n c) -> p n c", c=65)
                    rc = rcp.tile([P, 4], F32, tag="rc")
                    nc.vector.reciprocal(rc[:], fin_view[:, :, 64])
                    nc.vector.tensor_mul(
                        out_view[:, 4 * t:4 * t + 4, h, :],
                        fin_view[:, :, 0:64],
                        rc[:].broadcast_to([P, 4, HD]),
                    )

            # pair 0 projection, then attention with pair-1 projection
            # interleaved into the gaps (PE would otherwise idle on ACT).
            proj(0, qkT, 0, "act")
            proj(2, qkT, 2, "act")
            proj(4, vT, 0, "act")
            vtranspose(0)
            attn(0, 0)
            proj(1, qkT, 1, "dve")
            attn(0, 1)
            proj(3, qkT, 3, "dve")
            attn(0, 2)
            proj(5, vT, 1, "dve")
            attn(0, 3)
            vtranspose(1)
            for t in range(NT):
                attn(1, t)
                for b in range(4):
                    ib = 4 * t + b
                    nc.sync.dma_start(
                        o_d[ib * P:(ib + 1) * P, :],
                        out_sb[:, ib * HPC * HD:(ib + 1) * HPC * HD],
                    )


def build_program():
    nc = bacc.Bacc(
        "TRN2",
        target_bir_lowering=False,
        debug=False,
        enable_asserts=True,
    )
    x_d = nc.dram_tensor("x", [S, D], F32, kind="ExternalInput").ap()
    w_d = nc.dram_tensor("w", [D, COLS], F32R, kind="ExternalInput").ap()
    o_d = nc.dram_tensor("o", [S, HPC * HD], F32, kind="ExternalOutput").ap()

    with tile.TileContext(nc) as tc, ExitStack() as ctx:
        _build_body(ctx, tc, x_d, w_d, o_d)
    nc.compile()
    return nc


_CACHE = {}


def _compiled():
    if "nc" not in _CACHE:
        _CACHE["nc"] = build_program()
    return _CACHE["nc"]


def make_in_maps(x, w_qkv):
    x = np.asarray(x, dtype=np.float32)
    w_qkv = np.asarray(w_qkv, dtype=np.float32)
    in_maps = []
    for c in range(NCORES):
        b = c // 4
        cs = (c % 4) * HPC * HD
        w_slice = np.concatenate(
            [
                w_qkv[:, cs:cs + HPC * HD],
                w_qkv[:, D + cs:D + cs + HPC * HD],
                w_qkv[:, 2 * D + cs:2 * D + cs + HPC * HD],
            ],
            axis=1,
        )
        in_maps.append(
            {
                "x": np.ascontiguousarray(x[b]),
                "w": np.ascontiguousarray(w_slice),
            }
        )
    return in_maps


def gather_out(results):
    out = np.empty((B, S, D), np.float32)
    for c in range(NCORES):
        b = c // 4
        cs = (c % 4) * HPC * HD
        out[b][:, cs:cs + HPC * HD] = results[c]["o"]
    return out


def kernel(x, w_qkv, w_o=None, **_):
    nc = _compiled()
    res = run_bass_kernel_spmd(nc, make_in_maps(x, w_qkv), core_ids=list(range(NCORES)))
    return gather_out(res.results)
